# revision 45
# baseline (speedup 1.0000x reference)
"""Multi-head causal attention (B=4, T=2048, D=1024, H=16) on 8 TRN2 NeuronCores.

Sharding: data-parallel over batch (4) x tensor-parallel over heads (2 groups
of 8). Core c handles batch c//2, head-group c%2: its Q/K/V projections
(weight-column shards), causal attention for its 8 heads, and a partial
output projection (weight-row shard). The pairwise reduction of the two
partials per batch happens on host (4 cheap bf16 adds).

Datapath is bf16 (inputs converted host-side; PSUM accumulation stays fp32):
matmuls run at full PE rate at any moving size, input DMA halves, DVE mask
multiplies hit the 2x mode. Measured rel err ~4.5e-3 against the fp32
reference (gate 2e-2).

Schedule: chunk-major over (512-query chunk c, head-pair pr) "units" so the
scalar engine's exp stream (the #2 engine load, ~145us) starts ~13us in.
Projection / out-projection work is a (need, pull)-tagged filler list drained
between attention blocks to keep the PE (the bottleneck, ~228us busy) fed;
each round's out-projection is spread across the next round's units. The
causal diagonal runs at 128-query granularity: QK^T moving slices of
512/384/256/128 columns, one strided-AP exp covering just the two heads'
valid strips, AV accumulation windows shrunk to match, and a single shared
128x128 triangular mask applied only to true diagonal blocks. The AV lag
queue (LAG kb's between exp and AV) crosses unit boundaries so a unit's
start interleaves with the previous unit's AV matmuls.

Softmax normalization: denominators ride as a 65th V row through the AV
matmul, reciprocal_approx on DVE, partition-broadcast on the otherwise-idle
Pool engine (GPSIMD cannot touch PSUM, so all psum->sbuf copies stay on
DVE/Act), and the scale is fused into the psum->sbuf attention-output copy.
On the last unit each 128-query block is normalized and out-projected as
soon as its own diagonal AV lands, shortening the endgame drain.
"""

import sys

if "/opt/trn_rl_repo" not in sys.path:
    sys.path.insert(0, "/opt/trn_rl_repo")

import ml_dtypes
import numpy as np

import concourse.bass as bass
import concourse.mybir as mybir
from concourse import bacc
from concourse.bass import MemorySpace
from concourse.tile import TileContext

B, T, D = 4, 2048, 1024
H, DH = 16, 64
HG = 8          # heads per core
GW = HG * DH    # group width = 512
P = 128
KD = D // P     # 8 contraction chunks
NTB = T // P    # 16 key blocks of 128
N_CORES = 8
LAG = 6         # kb's between exp issue and AV consumption

F32 = mybir.dt.float32
BF16 = mybir.dt.bfloat16
U16 = mybir.dt.uint16


def build_nc():
    nc = bacc.Bacc()

    xq = nc.dram_tensor("xq", [D, T], BF16, kind="ExternalInput")
    xk = nc.dram_tensor("xk", [D, T], BF16, kind="ExternalInput")
    xv = nc.dram_tensor("xv", [D, T], BF16, kind="ExternalInput")
    wq = nc.dram_tensor("wq", [D, GW], BF16, kind="ExternalInput")
    wk = nc.dram_tensor("wk", [D, GW], BF16, kind="ExternalInput")
    wv = nc.dram_tensor("wv", [D, GW], BF16, kind="ExternalInput")
    wo = nc.dram_tensor("wo", [GW, D], BF16, kind="ExternalInput")
    msk = nc.dram_tensor("msk", [P, P], BF16, kind="ExternalInput")
    out = nc.dram_tensor("out", [T, D], BF16, kind="ExternalOutput")

    with TileContext(nc) as tc:
        cms = []

        def pool(name, bufs, space=None):
            kw = {"space": space} if space else {}
            cm = tc.tile_pool(name=name, bufs=bufs, **kw)
            cms.append(cm)
            return cm.__enter__()

        big = pool("big", 1)
        ppool = pool("pp", 9)
        xkq = pool("xkq", 8)
        xvp = pool("xvp", 4)
        rtp = pool("rtp", 2)
        bcp = pool("bcp", 2)
        obp = pool("obp", 3)
        sp = pool("sp", 2, MemorySpace.PSUM)    # [128,1024] f32 -> 2 banks x2
        avp = pool("avp", 2, MemorySpace.PSUM)  # [65,512] f32  -> 1 bank x2
        psp = pool("psp", 2, MemorySpace.PSUM)  # [128,512] f32 -> 1 bank x2

        kts = [big.tile([P, T], BF16, name=f"kt{j}") for j in range(4)]
        qts = [big.tile([P, T], BF16, name=f"qt{j}") for j in range(4)]
        # attention output reuses qt storage: qt[:, chunk] is dead after that
        # chunk's QK^T matmuls, exactly when the tail writes it
        aots = qts
        vsb = big.tile([P, NTB, HG * 65], BF16, name="vsb")
        wk_sba = big.tile([P, KD, 256], BF16, name="wk_sba")
        wk_sbb = big.tile([P, KD, 256], BF16, name="wk_sbb")
        wq_sba = big.tile([P, KD, 256], BF16, name="wq_sba")
        wq_sbb = big.tile([P, KD, 256], BF16, name="wq_sbb")
        wv_sb = big.tile([P, KD, GW], BF16, name="wv_sb")
        wo_sb = big.tile([P, 4, D], BF16, name="wo_sb")
        mask_sb = big.tile([P, P], BF16, name="mask_sb")

        vones = vsb.rearrange("p tb (h m) -> p tb h m", h=HG)[:, :, :, 64:65]
        nc.vector.memset(vones.bitcast(U16), 0x3F80)

        lo, hi = slice(0, 64), slice(64, 128)

        xk_t, xq_t, xv_t = {}, {}, {}

        def dma_x(src, store, ch, pool_, tag):
            t = pool_.tile([P, KD, 256], BF16, name=f"x{tag}", tag=tag)
            nc.sync.dma_start(
                t, src.rearrange("(ko p) t -> p ko t", p=P)[:, :, ch * 256:(ch + 1) * 256]
            )
            store[ch] = t

        # ---- upfront DMAs: half-column weight loads sequenced against the
        # x chunks so the PE starts ~4us in and never gaps (a gap re-triggers
        # the slow p-state ramp) ----
        wk_r = wk.rearrange("(ko p) j -> p ko j", p=P)
        wq_r = wq.rearrange("(ko p) j -> p ko j", p=P)
        nc.sync.dma_start(wk_sba, wk_r[:, :, 0:256])
        dma_x(xk, xk_t, 0, xkq, "xk")
        dma_x(xk, xk_t, 1, xkq, "xk")
        nc.sync.dma_start(wk_sbb, wk_r[:, :, 256:GW])
        nc.sync.dma_start(wq_sba, wq_r[:, :, 0:256])
        dma_x(xq, xq_t, 0, xkq, "xq")
        dma_x(xq, xq_t, 1, xkq, "xq")
        nc.sync.dma_start(mask_sb, msk[:, :])
        nc.sync.dma_start(wv_sb, wv.rearrange("(ko p) j -> p ko j", p=P))
        dma_x(xv, xv_t, 0, xvp, "xv")
        dma_x(xv, xv_t, 1, xvp, "xv")
        # wq second half last: its jb1 columns are already in the first half,
        # and jb2/jb3 are not needed until unit 2
        nc.sync.dma_start(wq_sbb, wq_r[:, :, 256:GW])

        # ---- filler steps: (need, pull, kind, fn) ----
        steps = []

        def kq_step(w_halves, xst, dst, ch, jb):
            def fn():
                w_sb = w_halves[jb // 2]
                jo = (jb % 2) * P
                ps = psp.tile([P, 256], F32, name="ps_kq", tag="ps")
                for kd in range(KD):
                    nc.tensor.matmul(
                        ps, w_sb[:, kd, jo:jo + P], xst[ch][:, kd, :],
                        start=(kd == 0), stop=(kd == KD - 1),
                    )
                nc.vector.tensor_copy(dst[jb][:, ch * 256:(ch + 1) * 256], ps)
            return fn

        def v_step(tb):
            def fn():
                xt = xv_t[tb // 2]
                co = (tb % 2) * P
                ps = psp.tile([P, GW], F32, name="ps_v", tag="ps")
                for kd in range(KD):
                    nc.tensor.matmul(
                        ps, xt[:, kd, co:co + P], wv_sb[:, kd, :],
                        start=(kd == 0), stop=(kd == KD - 1),
                    )
                nc.vector.tensor_copy(
                    vsb[:, tb, :].rearrange("p (h m) -> p h m", h=HG)[:, :, 0:64],
                    ps.rearrange("p (h m) -> p h m", h=HG),
                )
            return fn

        def o_step(tb, oc):
            def fn():
                if oc == 0:
                    ob_t[tb % 2] = obp.tile([P, D], BF16, name="ob", tag="ob")
                ob = ob_t[tb % 2]
                ps = psp.tile([P, GW], F32, name="ps_o", tag="ps")
                for jb in range(4):
                    nc.tensor.matmul(
                        ps, aots[jb][:, tb * P:(tb + 1) * P],
                        wo_sb[:, jb, oc * GW:(oc + 1) * GW],
                        start=(jb == 0), stop=(jb == 3),
                    )
                if tb >= 12:
                    nc.scalar.copy(ob[:, oc * GW:(oc + 1) * GW], ps)
                else:
                    nc.vector.tensor_copy(ob[:, oc * GW:(oc + 1) * GW], ps)
                if oc == 1:
                    nc.sync.dma_start(out[tb * P:(tb + 1) * P, :], ob)
            return fn

        ob_t = {}

        def wo_dma():
            nc.sync.dma_start(wo_sb, wo.rearrange("(jb p) o -> p jb o", p=P))

        v_idx, o_idx = {}, {}
        for u in range(16):
            r, pr = u // 4, u % 4
            if pr == 1 and r <= 2:
                # next round's x chunks: listed one round early so lookahead
                # pulls issue the DMAs well before the round boundary
                for ch in (2 * r + 2, 2 * r + 3):
                    steps.append((4 * r + 4, u - 1, "x",
                                  (lambda ch=ch: dma_x(xk, xk_t, ch, xkq, "xk"))))
                    steps.append((4 * r + 4, u - 1, "x",
                                  (lambda ch=ch: dma_x(xq, xq_t, ch, xkq, "xq"))))
                    steps.append((4 * r + 4, u - 1, "x",
                                  (lambda ch=ch: dma_x(xv, xv_t, ch, xvp, "xv"))))
            if u == 3:
                steps.append((u, 0, "x", wo_dma))
            if u == 0:
                # ordered to match DMA arrivals: chunk-0 jb0/jb1 (first wk
                # half), chunk-1 jb0/jb1, then jb2 (second half), then Q jb0
                for ch in (0, 1):
                    for jb in (0, 1):
                        steps.append((0, 0, "p", kq_step((wk_sba, wk_sbb), xk_t, kts, ch, jb)))
                for ch in (0, 1):
                    steps.append((0, 0, "p", kq_step((wk_sba, wk_sbb), xk_t, kts, ch, 2)))
                for ch in (0, 1):
                    steps.append((0, 0, "p", kq_step((wq_sba, wq_sbb), xq_t, qts, ch, 0)))
                for ch in (0, 1):
                    steps.append((1, 0, "p", kq_step((wk_sba, wk_sbb), xk_t, kts, ch, 3)))
            elif u < 4:
                for ch in (2 * r, 2 * r + 1):
                    steps.append((u, u - PULLOFF, "p",
                                  kq_step((wq_sba, wq_sbb), xq_t, qts, ch, pr)))
            else:
                # Q steps first: with diagonal-first kb order the unit's
                # opening QK reads the new kt chunk, so the K copies overlap
                # the remaining forced steps instead of gating the unit
                for ch in (2 * r, 2 * r + 1):
                    steps.append((u, u - PULLOFF, "p",
                                  kq_step((wq_sba, wq_sbb), xq_t, qts, ch, pr)))
                for ch in (2 * r, 2 * r + 1):
                    steps.append((u, u - PULLOFF, "p",
                                  kq_step((wk_sba, wk_sbb), xk_t, kts, ch, pr)))
            if pr == 0:
                for tb in range(4 * r, 4 * r + 4):
                    v_idx[tb] = len(steps)
                    steps.append((17, u - PULLOFF, "p", v_step(tb)))
            if pr in (1, 2, 3) and r >= 1:
                tbs = list(range(4 * (r - 1), 4 * r))
                grp = ({1: tbs[0:2], 2: tbs[2:3], 3: tbs[3:4]}[pr]
                       if r < 3 else
                       {1: tbs[0:1], 2: tbs[1:2], 3: tbs[2:4]}[pr])
                for tb in grp:
                    for oc in (0, 1):
                        steps.append((u, u, "o", o_step(tb, oc)))
        for tb in range(12, 16):
            for oc in (0, 1):
                o_idx[(tb, oc)] = len(steps)
                steps.append((18, 18, "o", o_step(tb, oc)))

        emitted = [False] * len(steps)
        head = [0]

        def emit_step(i):
            if not emitted[i]:
                emitted[i] = True
                steps[i][3]()

        def ensure_v(tb):
            for t in range(tb + 1):
                emit_step(v_idx[t])

        def drain_force(maxneed):
            while head[0] < len(steps) and emitted[head[0]]:
                head[0] += 1
            i = head[0]
            while i < len(steps):
                if not emitted[i] and steps[i][0] <= maxneed:
                    emitted[i] = True
                    steps[i][3]()
                elif not emitted[i] and steps[i][0] > maxneed + 4:
                    break
                i += 1

        def drain_pull(u, limit):
            while head[0] < len(steps) and emitted[head[0]]:
                head[0] += 1
            n, i = 0, head[0]
            scanned = 0
            while i < len(steps) and n < limit and scanned < 80:
                if not emitted[i] and steps[i][1] <= u:
                    emitted[i] = True
                    steps[i][3]()
                    n += 1
                scanned += 1
                i += 1

        # ---- attention units, chunk-major; AV lag queue crosses units ----
        scale = float(DH) ** -0.5
        pend = []       # (unit, kb, F, d0, pp)
        ctx = {}        # unit -> dict(av1, av2, pr, nblk, cs0)

        def emit_av(e):
            uu, kb, F, d0, pp, st, sop = e
            cx = ctx[uu]
            ensure_v(kb)
            pr_ = cx["pr"]
            nc.tensor.matmul(
                cx["av1"][:, d0:d0 + F],
                vsb[:, kb, (2 * pr_) * 65:(2 * pr_) * 65 + 65],
                pp[:, 0:F], start=st, stop=sop, skip_group_check=True,
            )
            nc.tensor.matmul(
                cx["av2"][:, d0:d0 + F],
                vsb[:, kb, (2 * pr_ + 1) * 65:(2 * pr_ + 1) * 65 + 65],
                pp[:, 512:512 + F], start=st, stop=sop, skip_group_check=True,
            )
            if uu == 15:
                # column block qb is final once its diagonal AV (j == qb)
                # lands: normalize + out-project it while later AVs run
                j = kb - 4 * 3
                if j >= 0:
                    emit_tail15_qb(cx, j)
            cx["left"] -= 1
            if cx["left"] == 0:
                emit_tail(uu)

        def emit_tail15_qb(cx, q4):
            av1, av2, aot, cs0 = cx["av1"], cx["av2"], cx["aot"], cx["cs0"]
            qs4 = slice(q4 * 128, q4 * 128 + 128)
            cs4 = slice(cs0 + q4 * 128, cs0 + q4 * 128 + 128)
            rt4 = rtp.tile([1, 256], F32, name="rt4", tag="rt4")
            nc.vector.tensor_copy(rt4[0:1, 0:128], av1[64:65, qs4])
            nc.vector.tensor_copy(rt4[0:1, 128:256], av2[64:65, qs4])
            nc.vector.reciprocal_approx_fast(rt4, rt4)
            bc4 = bcp.tile([P, 256], F32, name="bc4", tag="bc4")
            nc.gpsimd.partition_broadcast(bc4[:, 0:128], rt4[0:1, 0:128])
            nc.gpsimd.partition_broadcast(bc4[:, 128:256], rt4[0:1, 128:256])
            nc.vector.tensor_mul(aot[lo, cs4], av1[0:64, qs4], bc4[0:64, 0:128])
            nc.vector.tensor_mul(aot[hi, cs4], av2[0:64, qs4], bc4[64:128, 128:256])
            emit_step(o_idx[(12 + q4, 0)])
            emit_step(o_idx[(12 + q4, 1)])

        def emit_tail(uu):
            cx = ctx.pop(uu)
            av1, av2, aot, cs0 = cx["av1"], cx["av2"], cx["aot"], cx["cs0"]
            if uu < 15:
                rt = rtp.tile([1, 1024], F32, name="rt", tag="rt")
                nc.vector.tensor_copy(rt[0:1, 0:512], av1[64:65, :])
                nc.vector.tensor_copy(rt[0:1, 512:1024], av2[64:65, :])
                nc.vector.reciprocal_approx_fast(rt, rt)
                bc = bcp.tile([P, 1024], F32, name="bc", tag="bc")
                nc.gpsimd.partition_broadcast(bc[:, 0:512], rt[0:1, 0:512])
                nc.gpsimd.partition_broadcast(bc[:, 512:1024], rt[0:1, 512:1024])
                cs = slice(cs0, cs0 + 512)
                nc.vector.tensor_mul(aot[lo, cs], av1[0:64, :], bc[0:64, 0:512])
                nc.vector.tensor_mul(aot[hi, cs], av2[0:64, :], bc[64:128, 512:1024])
            else:
                pass  # handled per-qb in emit_tail15_qb

        for u in range(16):
            c, pr = u // 4, u % 4
            kt, qt, aot = kts[pr], qts[pr], aots[pr]
            nblk = 4 * (c + 1)
            cs0 = 512 * c
            drain_force(u)

            av1 = avp.tile([65, 512], F32, name="av1", tag="av")
            av2 = avp.tile([65, 512], F32, name="av2", tag="av")
            ctx[u] = {"av1": av1, "av2": av2, "pr": pr, "nblk": nblk,
                      "cs0": cs0, "aot": aot, "left": nblk}

            if u < 15:
                kb_order = list(range(4 * c, nblk)) + list(range(0, 4 * c))
            else:
                kb_order = list(range(nblk))
            for ki, kb in enumerate(kb_order):
                j = kb - 4 * c
                F = 512 if j < 0 else 512 - 128 * j
                d0 = 0 if j < 0 else 128 * j
                s_pair = sp.tile([P, 1024], F32, name="s_pair", tag="sp")
                ks = slice(kb * P, (kb + 1) * P)
                qs = slice(cs0 + d0, cs0 + d0 + F)
                nc.tensor.matmul(
                    s_pair[:, 0:F], kt[lo, ks], qt[lo, qs], start=True, stop=True,
                )
                nc.tensor.matmul(
                    s_pair[:, 512:512 + F], kt[hi, ks], qt[hi, qs],
                    start=True, stop=True,
                )
                pp = ppool.tile([P, 1024], BF16, name="p_pair", tag="pp")
                sv = s_pair.rearrange("p (h q) -> p h q", h=2)[:, :, 0:F]
                pv = pp.rearrange("p (h q) -> p h q", h=2)[:, :, 0:F]
                nc.scalar.activation(
                    pv, sv, mybir.ActivationFunctionType.Exp, scale=scale,
                )
                if j >= 0:
                    # mask the 128x128 diagonal block (first 128 cols of strip)
                    nc.vector.tensor_mul(pp[:, 0:128], pp[:, 0:128], mask_sb)
                    nc.vector.tensor_mul(pp[:, 512:640], pp[:, 512:640], mask_sb)
                drain_pull(u, 3)
                pend.append((u, kb, F, d0, pp))
                if len(pend) > LAG:
                    emit_av(pend.pop(0))
                if u == 15 and kb >= 12:
                    # drain the lag queue faster at the very end so the
                    # per-qb tails + final out-projection start earlier
                    for _ in range(2):
                        if pend:
                            emit_av(pend.pop(0))

        while pend:
            emit_av(pend.pop(0))
            drain_pull(16, 1)
        drain_force(99)

        for cm in reversed(cms):
            cm.__exit__(None, None, None)

    nc.finalize()
    return nc


def _to_bf16(a):
    return np.ascontiguousarray(a).astype(ml_dtypes.bfloat16)


def make_in_maps(q, k, v, Wq, Wk, Wv, Wo):
    mask_bf = (
        np.arange(P)[None, :] >= np.arange(P)[:, None]
    ).astype(ml_dtypes.bfloat16)
    xs = [
        {"xq": _to_bf16(q[b].T), "xk": _to_bf16(k[b].T), "xv": _to_bf16(v[b].T)}
        for b in range(B)
    ]
    ws = []
    for g in range(2):
        hs = slice(g * GW, (g + 1) * GW)
        ws.append({
            "wq": _to_bf16(Wq[hs, :].T),
            "wk": _to_bf16(Wk[hs, :].T),
            "wv": _to_bf16(Wv[hs, :].T),
            "wo": _to_bf16(Wo[:, hs].T),
        })
    return [
        {**xs[c // 2], **ws[c % 2], "msk": mask_bf} for c in range(N_CORES)
    ]


_NC_CACHE = None


def kernel(q, k, v, mask, Wq, Wk, Wv, Wo):
    global _NC_CACHE
    if _NC_CACHE is None:
        _NC_CACHE = build_nc()
    nc = _NC_CACHE

    from concourse.bass_utils import run_bass_kernel_spmd

    q, k, v = np.asarray(q), np.asarray(k), np.asarray(v)
    Wq, Wk, Wv, Wo = (np.asarray(t) for t in (Wq, Wk, Wv, Wo))
    in_maps = make_in_maps(q, k, v, Wq, Wk, Wv, Wo)

    r = run_bass_kernel_spmd(nc, in_maps, core_ids=list(range(N_CORES)))
    parts = [np.asarray(r.results[c]["out"], dtype=np.float32) for c in range(N_CORES)]
    y = np.stack([parts[2 * b] + parts[2 * b + 1] for b in range(B)])
    return y
